# revision 5
# baseline (speedup 1.0000x reference)
"""MultiHeadAttention TRN2 kernel: B=2, S=2048, D=1024, H=16, DK=64, 8 cores.

Sharding: core c handles batch b=c//4 and heads hg=(c%4)*4 .. +3 (data + head
parallel). Projections are column-split by head; out-proj row-split; the
all-reduce after out-proj is done on host (sum of 4 partials per batch).

Device dataflow (per core, all matmuls float32r = full PE rate):
  qT/kT = (wT-slice).T @ QT/KT          -> (128=2 heads, S) per head-pair
  v     = VT.T @ wvT-slice (+ ones col) -> natural (k-rows, 65) chunks
  scoresT[k,q] = kT.T-chunk @ qT        (K=64 packed pairs at partitions 0/64)
  expT = exp(scoresT/8)                 (ScalarE, the throughput floor)
  ctxU^T[f,q] (+den row) = v_aug.T @ expT  (psum accumulate over k-chunks)
  ctx^T = ctxU^T * (1/den)              (shuffle-broadcast + recip_approx)
  partial_out = ctx^T.T @ woT-slice     -> (S, 1024) partial, summed on host

Bias handling (exact): bq added on device (per-partition add in qT layout);
bk dropped (softmax shift-invariance); bv and bo folded on host as
out += bv @ wo.T + bo (softmax weights sum to 1).
"""

from contextlib import ExitStack

import numpy as np

B, S, D, H, DK = 2, 2048, 1024, 16, 64
NCORES = 8
HPC = H // (NCORES // B)      # heads per core = 4
R = HPC * DK                  # local feats = 256
NKC = S // 128                # k-chunks = 16
VW = 65                       # v chunk width (64 + ones col)

_CACHE = {}


def _build():
    import concourse.mybir as mybir
    import concourse.tile as tile
    from concourse import bacc

    f32 = mybir.dt.float32
    f32r = mybir.dt.float32r
    Exp = mybir.ActivationFunctionType.Exp
    Add = mybir.AluOpType.add

    nc = bacc.Bacc(
        "TRN2", target_bir_lowering=False, debug=False,
        enable_asserts=True, num_devices=NCORES,
    )

    QT_d = nc.dram_tensor("QT", [D, S], f32r, kind="ExternalInput").ap()
    KT_d = nc.dram_tensor("KT", [D, S], f32r, kind="ExternalInput").ap()
    VT_d = nc.dram_tensor("VT", [D, S], f32r, kind="ExternalInput").ap()
    wqT_d = nc.dram_tensor("wqT", [D, R], f32r, kind="ExternalInput").ap()
    wkT_d = nc.dram_tensor("wkT", [D, R], f32r, kind="ExternalInput").ap()
    wvT_d = nc.dram_tensor("wvT", [D, R], f32r, kind="ExternalInput").ap()
    woT_d = nc.dram_tensor("woT", [R, D], f32r, kind="ExternalInput").ap()
    bq_d = nc.dram_tensor("bq", [R, 1], f32, kind="ExternalInput").ap()
    out_d = nc.dram_tensor("OUT", [S, D], f32, kind="ExternalOutput").ap()

    with tile.TileContext(nc) as tc, ExitStack() as ctx:
        sb = ctx.enter_context(tc.tile_pool(name="sb", bufs=1))
        qkv_in = ctx.enter_context(tc.tile_pool(name="qkv_in", bufs=10))
        expp = ctx.enter_context(tc.tile_pool(name="expp", bufs=3))
        normp = ctx.enter_context(tc.tile_pool(name="normp", bufs=4))
        osb = ctx.enter_context(tc.tile_pool(name="osb", bufs=3))

        # ---- persistent weights ----
        wq_sb = sb.tile([128, 8 * R], f32r)   # D-chunk d at cols [R*d : R*(d+1)]
        wk_sb = sb.tile([128, 8 * R], f32r)
        wv_sb = sb.tile([128, 8 * R], f32r)
        for d in range(8):
            nc.sync.dma_start(wq_sb[:, R * d:R * (d + 1)], wqT_d[128 * d:128 * (d + 1), :])
            nc.sync.dma_start(wk_sb[:, R * d:R * (d + 1)], wkT_d[128 * d:128 * (d + 1), :])
            nc.sync.dma_start(wv_sb[:, R * d:R * (d + 1)], wvT_d[128 * d:128 * (d + 1), :])
        wo_sb = [sb.tile([128, D], f32r, name=f"wo_sb{cn}") for cn in range(2)]
        for cn in range(2):
            nc.sync.dma_start(wo_sb[cn][:], woT_d[128 * cn:128 * (cn + 1), :])
        bq_sb = sb.tile([128, 2], f32)
        for hp in range(2):
            nc.sync.dma_start(bq_sb[:, hp:hp + 1], bq_d[128 * hp:128 * (hp + 1), :])

        # ---- persistent activations ----
        qT_sb = [sb.tile([128, S], f32r, name=f"qT_sb{hp}") for hp in range(2)]
        kT_sb = [sb.tile([128, S], f32r, name=f"kT_sb{hp}") for hp in range(2)]
        v_all = sb.tile([128, HPC * NKC * VW], f32r)  # head h chunk c at cols (h*NKC+c)*VW
        ctxT_sb = [sb.tile([128, S], f32r, name=f"ctxT_sb{cn}") for cn in range(2)]

        onecol = sb.tile([128, 1], f32)
        nc.vector.memset(onecol[:], 1.0)
        vv = v_all.rearrange("p (n c) -> p n c", c=VW)[:, :, 64:65].rearrange(
            "p n c -> p (n c)")
        nc.vector.tensor_copy(vv, onecol[:].broadcast_to((128, HPC * NKC)))

        # ---- phase A: projections ----
        with tc.tile_pool(name="projp", bufs=2, space="PSUM") as projp:
            # qT then kT (attention scores can start as soon as these finish)
            for name, src, w_sb, q_out, bias in (
                ("q", QT_d, wq_sb, qT_sb, True),
                ("k", KT_d, wk_sb, kT_sb, False),
            ):
                for sblk in range(4):
                    ins = []
                    for d in range(8):
                        t = qkv_in.tile([128, 512], f32r, name="qk_in")
                        nc.sync.dma_start(
                            t[:], src[128 * d:128 * (d + 1), 512 * sblk:512 * (sblk + 1)])
                        ins.append(t)
                    for hp in range(2):
                        p_ps = projp.tile([128, 512], mybir.dt.float32, name="p_ps")
                        for d in range(8):
                            nc.tensor.matmul(
                                p_ps[:],
                                w_sb[:, R * d + 128 * hp:R * d + 128 * (hp + 1)],
                                ins[d][:], start=(d == 0), stop=(d == 7))
                        dst = q_out[hp][:, 512 * sblk:512 * (sblk + 1)]
                        if bias:
                            nc.vector.tensor_scalar(
                                dst, p_ps[:], bq_sb[:, hp:hp + 1], None, op0=Add)
                        else:
                            nc.vector.tensor_copy(dst, p_ps[:])
            # v projection (natural layout + strided scatter into v_all)
            for sblk in range(4):
                ins = []
                for d in range(8):
                    t = qkv_in.tile([128, 512], f32r, name="v_in")
                    nc.sync.dma_start(
                        t[:], VT_d[128 * d:128 * (d + 1), 512 * sblk:512 * (sblk + 1)])
                    ins.append(t)
                for sub in range(4):
                    c = 4 * sblk + sub
                    v_ps = projp.tile([128, R], mybir.dt.float32, name="v_ps")
                    for d in range(8):
                        nc.tensor.matmul(
                            v_ps[:], ins[d][:, 128 * sub:128 * (sub + 1)],
                            wv_sb[:, R * d:R * (d + 1)], start=(d == 0), stop=(d == 7))
                    # psum (128, 4*64 heads side by side) -> v_all strided per head
                    va = v_all.rearrange("p (h n c) -> p h n c", h=HPC, n=NKC)
                    nc.vector.tensor_copy(
                        va[:, :, c:c + 1, 0:64],
                        v_ps[:].rearrange("p (h n c) -> p h n c", h=HPC, n=1))

        # ---- phase B: attention ----
        with tc.tile_pool(name="scorep", bufs=1, space="PSUM") as scorep, \
             tc.tile_pool(name="ctxp", bufs=1, space="PSUM") as ctxp:
            for qh in range(2):
                for hp in range(2):
                    c_ps = [ctxp.tile([128, 512], mybir.dt.float32, name=f"c_ps{h}{qb}")
                            for h in range(2) for qb in range(2)]
                    for c in range(NKC):
                        s_ps = [scorep.tile([128, 1024], mybir.dt.float32, name=f"s_ps{h}")
                                for h in range(2)]
                        for h in range(2):
                            for qb in range(2):
                                nc.tensor.matmul(
                                    s_ps[h][:, 512 * qb:512 * (qb + 1)],
                                    kT_sb[hp][64 * h:64 * (h + 1), 128 * c:128 * (c + 1)],
                                    qT_sb[hp][64 * h:64 * (h + 1),
                                              1024 * qh + 512 * qb:1024 * qh + 512 * (qb + 1)],
                                    start=True, stop=True)
                        for h in range(2):
                            expT = expp.tile([128, 1024], f32r, name="expT")
                            nc.scalar.activation(expT[:], s_ps[h][:], Exp, scale=0.125)
                            gh = 2 * hp + h
                            for qb in range(2):
                                nc.tensor.matmul(
                                    c_ps[2 * h + qb][0:VW, :],
                                    v_all[:, (gh * NKC + c) * VW:(gh * NKC + c + 1) * VW],
                                    expT[:, 512 * qb:512 * (qb + 1)],
                                    start=(c == 0), stop=(c == NKC - 1))
                    for h in range(2):
                        for qb in range(2):
                            rin = normp.tile([32, 512], f32, name="rin")
                            nc.vector.stream_shuffle(
                                rin[:], c_ps[2 * h + qb][64:96, :], [0] * 32)
                            rb = normp.tile([32, 512], f32, name="rb")
                            nc.vector.reciprocal_approx_fast(out=rb[:], in_=rin[:])
                            for hf in range(2):
                                nc.vector.tensor_mul(
                                    ctxT_sb[hp][64 * h + 32 * hf:64 * h + 32 * (hf + 1),
                                                1024 * qh + 512 * qb:1024 * qh + 512 * (qb + 1)],
                                    c_ps[2 * h + qb][32 * hf:32 * (hf + 1), :],
                                    rb[:])

        # ---- phase C: out projection ----
        with tc.tile_pool(name="outp", bufs=2, space="PSUM") as outp:
            for sc in range(16):
                o_ps = outp.tile([128, D], mybir.dt.float32, name="o_ps")
                for nb in range(2):
                    for cn in range(2):
                        nc.tensor.matmul(
                            o_ps[:, 512 * nb:512 * (nb + 1)],
                            ctxT_sb[cn][:, 128 * sc:128 * (sc + 1)],
                            wo_sb[cn][:, 512 * nb:512 * (nb + 1)],
                            start=(cn == 0), stop=(cn == 1))
                o_sb = osb.tile([128, D], f32, name="o_sb")
                nc.vector.tensor_copy(o_sb[:], o_ps[:])
                nc.sync.dma_start(out_d[128 * sc:128 * (sc + 1), :], o_sb[:])

    nc.compile()
    return nc


def kernel(Q, K, V, wq, bq, wk, bk, wv, bv, wo, bo):
    from concourse.bass_utils import run_bass_kernel_spmd

    if "nc" not in _CACHE:
        _CACHE["nc"] = _build()
    nc = _CACHE["nc"]

    Q = np.asarray(Q, np.float32)
    K = np.asarray(K, np.float32)
    V = np.asarray(V, np.float32)
    QT = [np.ascontiguousarray(Q[b].T) for b in range(B)]
    KT = [np.ascontiguousarray(K[b].T) for b in range(B)]
    VT = [np.ascontiguousarray(V[b].T) for b in range(B)]
    wqT = [np.ascontiguousarray(np.asarray(wq, np.float32)[g * R:(g + 1) * R].T)
           for g in range(4)]
    wkT = [np.ascontiguousarray(np.asarray(wk, np.float32)[g * R:(g + 1) * R].T)
           for g in range(4)]
    wvT = [np.ascontiguousarray(np.asarray(wv, np.float32)[g * R:(g + 1) * R].T)
           for g in range(4)]
    woT = [np.ascontiguousarray(np.asarray(wo, np.float32)[:, g * R:(g + 1) * R].T)
           for g in range(4)]
    bqs = [np.ascontiguousarray(np.asarray(bq, np.float32)[g * R:(g + 1) * R, None])
           for g in range(4)]

    in_maps = []
    for c in range(NCORES):
        b, g = c // 4, c % 4
        in_maps.append({
            "QT": QT[b], "KT": KT[b], "VT": VT[b],
            "wqT": wqT[g], "wkT": wkT[g], "wvT": wvT[g], "woT": woT[g],
            "bq": bqs[g],
        })

    global _LAST_IN_MAPS
    _LAST_IN_MAPS = in_maps
    res = run_bass_kernel_spmd(nc, in_maps, core_ids=list(range(NCORES)))

    host_bias = (np.asarray(bv, np.float32) @ np.asarray(wo, np.float32).T
                 + np.asarray(bo, np.float32))
    out = np.zeros((B, S, D), np.float32)
    for c in range(NCORES):
        out[c // 4] += res.results[c]["OUT"]
    out += host_bias[None, None, :]
    return out
